# revision 39
# baseline (speedup 1.0000x reference)
"""Transformer block (pre-LN attention + FFN) on 8 TRN2 NeuronCores — v3.

Sharding: cores 0-3 handle batch 0, cores 4-7 batch 1. Core c (b=c//4,
j=c%4) owns heads [4j:4j+4) of batch b for attention, and a strided token
shard for LN2/FFN/residual: rows {512r + 128j + [0,128) : r=0..3}.

Design notes:
  - LN1 replicated per batch-group (no AllGather). LN gamma/beta folded
    into Wq/Wk/Wv/W1 host-side; on-chip LN is pure (x-mu)*rstd, split
    into two single-scalar ops (the fused 2-AP-scalar tensor_scalar hits
    a ~16x slower DVE path).
  - QKV and attn*V in fp8e4m3 DoubleRow (weights prescaled x64). Scores,
    proj, FFN in bf16 (fp8 there breaks the 2e-2 budget).
  - Scores processed in 512-query chunks; attn*V computed TRANSPOSED
    (out[feat, query], stationary v, DR over the two 128-key subtiles):
    80 AV matmuls instead of 288, and no attnT transposes. Softmax
    normalization applies via reciprocal of the PSUM sum row + a K=1
    ones-matmul partition-broadcast + one multiply.
  - Every matmul costs ~126ns serial LDWEIGHTS (walrus pins ldw-opt
    off), so the kernel favors few, large (N=512) matmuls everywhere.
  - proj ReduceScatter chunked x4 and launched mid-attention; gpsimd is
    kept free of bulk work so collective triggers fire immediately.
"""

import numpy as np

import concourse.bass as bass
import concourse.mybir as mybir
import concourse.tile as tile
from concourse import bacc
from concourse.bass_utils import run_bass_kernel_spmd
from concourse.masks import make_identity
import concourse.bacc as _bacc_mod

_ONE_TABLE = "natural_log_exp_and_others"
if not getattr(_bacc_mod, "_act_table_pinned", False):
    _orig_get_tables = _bacc_mod.get_activation_tables

    def _pinned_tables(arch):
        t = _orig_get_tables(arch)
        return {k: (v if k == _ONE_TABLE else set()) for k, v in t.items()}

    _bacc_mod.get_activation_tables = _pinned_tables
    _bacc_mod._act_table_pinned = True

P = 128
C = 1024           # n_embd
KT = C // P        # 8 c-tiles
T = 2048           # tokens per batch
IT = T // P        # 16 token tiles
NH = 4             # heads per core
D = 64             # head dim
DL = NH * D        # 256 local head features
FF = 4096
FMT = FF // P      # 32 ffn m-tiles
CH2 = 512          # attention q-chunk
QC2 = T // CH2     # 4 q-chunks
T_OWN = 512        # own tokens per core (4 strided chunks of 128)
NCHUNK = 4         # RS chunks (512 global tokens each -> 128 own rows)
EPS = 1e-5
SCALE = 1.0 / 32.0   # C ** -0.5 (reference quirk)
WS = 64.0            # fp8 weight prescale
GROUPS = [[0, 1, 2, 3], [4, 5, 6, 7]]
NCORES = 8

f32 = mybir.dt.float32
f32r = mybir.dt.float32r
bf16 = mybir.dt.bfloat16
f8 = mybir.dt.float8e4
AX = mybir.AxisListType
ALU = mybir.AluOpType
ACT_F = mybir.ActivationFunctionType
DR = mybir.MatmulPerfMode.DoubleRow

NP_BF16 = mybir.dt.np(bf16)
NP_F8 = mybir.dt.np(f8)


def build(debug=False):
    nc = bacc.Bacc("TRN2", target_bir_lowering=False, debug=False,
                   num_devices=NCORES)
    _build_graph(nc)
    nc.compile()
    return nc



def _emit_proj_rs(nc, st, ps, attnT, wp_sb, rs_in, rs_out, r):
    for mt in range(4 * r, 4 * r + 4):
        ob = st.tile([P, C], bf16, tag="projev", bufs=2,
                     name=f"projev_{mt}")
        for n in range(2):
            pp = ps.tile([P, 512], f32, tag="big", bufs=2,
                         name=f"proj_{mt}_{n}")
            for kt2 in range(2):
                nc.tensor.matmul(
                    pp[:], attnT[:, kt2, mt * P:(mt + 1) * P],
                    wp_sb[:, kt2, n * 512:(n + 1) * 512],
                    start=(kt2 == 0), stop=(kt2 == 1))
            nc.vector.tensor_copy(
                out=ob[:, n * 512:(n + 1) * 512], in_=pp[:])
        nc.sync.dma_start(
            rs_in[r][(mt - 4 * r) * P:(mt - 4 * r + 1) * P, :],
            ob[:])
    nc.gpsimd.collective_compute(
        "ReduceScatter", ALU.add, ins=[rs_in[r].opt()],
        outs=[rs_out[r].opt()], replica_groups=GROUPS)

def _build_graph(nc):
    x_ext = nc.dram_tensor("x", [T, C], bf16, kind="ExternalInput").ap()
    xo_ext = nc.dram_tensor("xo", [T_OWN, C], bf16, kind="ExternalInput").ap()
    wq_ext = nc.dram_tensor("wq", [P, KT * DL], f8, kind="ExternalInput").ap()
    wk_ext = nc.dram_tensor("wk", [P, KT * DL], f8, kind="ExternalInput").ap()
    wv_ext = nc.dram_tensor("wv", [P, KT * DL], f8, kind="ExternalInput").ap()
    wp_ext = nc.dram_tensor("wp", [2, P, C], bf16, kind="ExternalInput").ap()
    w1_ext = nc.dram_tensor("w1", [FMT, P, KT * P], bf16,
                            kind="ExternalInput").ap()
    w2_ext = nc.dram_tensor("w2", [2, P, FMT * 512], bf16,
                            kind="ExternalInput").ap()
    bqk_ext = nc.dram_tensor("bqk", [P, 4], f32, kind="ExternalInput").ap()
    b1_ext = nc.dram_tensor("b1r", [P, FMT], f32, kind="ExternalInput").ap()
    bpb2_ext = nc.dram_tensor("bpb2", [2, C], f32, kind="ExternalInput").ap()
    out_ext = nc.dram_tensor("out", [T_OWN, C], f32, kind="ExternalOutput").ap()

    with tile.TileContext(nc) as tc:
        with (
            tc.tile_pool(name="sb", bufs=1) as sb,
            tc.tile_pool(name="st", bufs=3) as st,
            tc.tile_pool(name="ps", bufs=1, space="PSUM") as ps,
            tc.tile_pool(name="dram", bufs=1, space="DRAM") as dram,
        ):
            # ---- x load first (longest dependency chain) ----
            x_sb = sb.tile([P, IT, C], bf16, tag="xfull", name="x_sb")
            for i in range(IT):
                nc.sync.dma_start(x_sb[:, i, :], x_ext[i * P:(i + 1) * P, :])
            bnst = sb.tile([P, IT, 2, 6], f32, name="ln1_bnst")
            mv = sb.tile([P, IT, 2], f32, name="ln1_mv")   # (mean, var)
            rstd = sb.tile([P, IT], f32, name="ln1_rstd")
            musq = sb.tile([P, IT], f32, name="ln1_musq")

            # ---- constants ----
            id_bf = sb.tile([P, P], bf16)
            make_identity(nc, id_bf[:])
            ones_bf = sb.tile([P, D], bf16, name="ones_bf")
            nc.vector.memset(ones_bf[:], 1.0)
            eps_t = sb.tile([P, 1], f32, name="eps_t")
            nc.vector.memset(eps_t[:], EPS)

            b2_t = sb.tile([P, C], f32, name="b2_t")
            nc.sync.dma_start(b2_t[:], bpb2_ext[1][None, :].to_broadcast([P, C]))
            b2_r = b2_t[:]
            bqk_sb = sb.tile([P, 4], f32, name="bqk")   # [p, (bq0,bq1,bk0,bk1)]
            nc.sync.dma_start(bqk_sb[:], bqk_ext)
            b1_sb = sb.tile([P, FMT], f32, name="b1r")
            nc.sync.dma_start(b1_sb[:], b1_ext)

            # causal masks for the 4 key-128-blocks of a diagonal 512-chunk:
            # mask_m[p, hdup, y] = 1 where key (128*m + p) <= query y
            masks = []
            for m_i in range(4):
                m = sb.tile([P, 2, CH2], f8, name=f"mask{m_i}")
                nc.gpsimd.memset(m[:], 1.0)
                nc.gpsimd.affine_select(
                    out=m[:], in_=m[:], compare_op=ALU.is_ge, fill=0.0,
                    base=-128 * m_i, pattern=[[0, 2], [1, CH2]],
                    channel_multiplier=-1)
                masks.append(m)

            # ---- QKV weights (fp8, host-prearranged [kp, kt, d]) ----
            wqkv = sb.tile([P, 3, KT, DL], f8, name="wqkv")
            for wi, ext in enumerate((wq_ext, wk_ext, wv_ext)):
                nc.sync.dma_start(
                    wqkv[:, wi], ext.rearrange("p (kt d) -> p kt d", kt=KT))
            wq_sb, wk_sb, wv_sb = wqkv[:, 0], wqkv[:, 1], wqkv[:, 2]
            wp_sb = sb.tile([P, 2, C], bf16, name="wp")
            nc.sync.dma_start(wp_sb[:], wp_ext.rearrange("k p c -> p k c"))

            hT = sb.tile([P, KT, T], f8, name="hT")
            qT = sb.tile([P, 2, T], bf16, name="qT")
            kT_lo = sb.tile([P, 2, T], bf16, name="kT_lo")
            kT_hi = sb.tile([P, 2, T], bf16, name="kT_hi")
            v_aug = sb.tile([P, 2 * IT // 2, NH, P], f8, name="v_aug")
            nc.gpsimd.memset(kT_lo[64:128, :, :], 0.0)
            nc.gpsimd.memset(kT_hi[0:64, :, :], 0.0)
            nc.gpsimd.memset(v_aug[:, :, :, D + 1:], 0.0)
            nc.gpsimd.memset(v_aug[:, :, :, D:D + 1], 1.0)

            for r in range(4):
                sl4 = slice(4 * r, 4 * r + 4)
                for ii in range(4):
                    i = 4 * r + ii
                    for hh in range(2):
                        nc.vector.bn_stats(
                            bnst[:, i, hh, :],
                            x_sb[:, i, hh * 512:(hh + 1) * 512])
                    nc.vector.bn_aggr(mv[:, i, :], bnst[:, i, :, :])
                # rstd = exp(-0.5*ln(var+eps)); avoids sqrt (act-table swap)
                nc.scalar.activation(rstd[:, sl4], mv[:, sl4, 1], ACT_F.Ln,
                                     bias=eps_t[:])
                nc.scalar.activation(rstd[:, sl4], rstd[:, sl4], ACT_F.Exp,
                                     bias=0.0, scale=-0.5)
                nc.vector.tensor_mul(out=musq[:, sl4], in0=mv[:, sl4, 0],
                                     in1=rstd[:, sl4])
                nc.vector.tensor_scalar_mul(musq[:, sl4], musq[:, sl4], -1.0)
                # normalize on ACT: Identity(x*rstd - mu*rstd)
                for ii in range(4):
                    i = 4 * r + ii
                    nc.scalar.activation(
                        x_sb[:, i, :], x_sb[:, i, :], ACT_F.Identity,
                        bias=musq[:, i:i + 1], scale=rstd[:, i:i + 1])
                # transpose quarter -> hT (merged evacuation per i-tile)
                for ii in range(4):
                    i = 4 * r + ii
                    tp = ps.tile([P, KT * P], bf16, tag="mid", bufs=2,
                                 name=f"tp_h_{i}")
                    for ct in range(KT):
                        nc.tensor.transpose(tp[:, ct * P:(ct + 1) * P],
                                            x_sb[:, i, ct * P:(ct + 1) * P],
                                            id_bf[:])
                    nc.vector.tensor_copy(
                        out=hT[:, :, i * P:(i + 1) * P],
                        in_=tp[:].rearrange("p (ct x) -> p ct x", x=P))
                # q, k for this quarter (fp8 DoubleRow, N=512)
                tsl = slice(r * T_OWN, (r + 1) * T_OWN)
                for wi, w in enumerate((wq_sb, wk_sb)):
                    for mt in range(2):
                        pp = ps.tile([P, T_OWN], f32, tag="big", bufs=2,
                                     name=f"qk_{r}_{wi}_{mt}")
                        for kp in range(KT // 2):
                            nc.tensor.matmul(
                                pp[:], w[:, 2 * kp:2 * kp + 2,
                                         mt * P:(mt + 1) * P],
                                hT[:, 2 * kp:2 * kp + 2, tsl],
                                start=(kp == 0), stop=(kp == KT // 2 - 1),
                                perf_mode=DR)
                        bias = bqk_sb[:, 2 * wi + mt:2 * wi + mt + 1]
                        if wi == 0:
                            nc.vector.tensor_scalar(
                                out=qT[:, mt, tsl], in0=pp[:],
                                scalar1=1.0 / WS, scalar2=bias,
                                op0=ALU.mult, op1=ALU.add)
                        else:
                            nc.vector.tensor_scalar(
                                out=kT_lo[0:64, mt, tsl], in0=pp[0:64, :],
                                scalar1=1.0 / WS, scalar2=bias[0:64],
                                op0=ALU.mult, op1=ALU.add)
                            nc.vector.tensor_scalar(
                                out=kT_hi[64:128, mt, tsl], in0=pp[64:128, :],
                                scalar1=1.0 / WS, scalar2=bias[64:128],
                                op0=ALU.mult, op1=ALU.add)
                # v for this quarter (fp8 DoubleRow, out [tokens, feats])
                for tt in range(4):
                    stt = 4 * r + tt
                    pp = ps.tile([P, DL], f32, tag="big", bufs=2,
                                 name=f"v_{stt}")
                    for kp in range(KT // 2):
                        nc.tensor.matmul(
                            pp[:],
                            hT[:, 2 * kp:2 * kp + 2, stt * P:(stt + 1) * P],
                            wv_sb[:, 2 * kp:2 * kp + 2, :],
                            start=(kp == 0), stop=(kp == KT // 2 - 1),
                            perf_mode=DR)
                    nc.vector.tensor_scalar(
                        out=v_aug[:, stt, :, 0:D],
                        in0=pp[:].rearrange("p (h d) -> p h d", d=D),
                        scalar1=1.0 / WS, scalar2=None, op0=ALU.mult)

            # ---- attention (512-query chunks; AV transposed + DR) ----
            attnT = sb.tile([P, 2, T], bf16, name="attnT")
            rs_in = [dram.tile([CH2, C], bf16, name=f"rs_in_{r}")
                     for r in range(NCHUNK)]
            rs_out = [dram.tile([P, C], bf16, name=f"rs_out_{r}")
                      for r in range(NCHUNK)]
            x_own = sb.tile([P, NCHUNK, C], bf16, name="x_own")
            for r in range(NCHUNK):
                nc.sync.dma_start(x_own[:, r, :], xo_ext[r * P:(r + 1) * P, :])

            for qc in range(QC2):
                qsl = slice(qc * CH2, (qc + 1) * CH2)
                for hp in range(2):
                    av = [ps.tile([P, CH2], f32, tag="mid", bufs=2,
                                  name=f"av_{hp}_{qc}_{hl}")
                          for hl in range(2)]
                    for kc in range(2 * qc + 2):
                        ex = st.tile([P, 2, 2, CH2], f8, tag="expT", bufs=3,
                                     name=f"ex_{hp}_{qc}_{kc}")
                        for sh in range(2):
                            sc = ps.tile([P, 2, CH2], f32, tag="quad", bufs=2,
                                         name=f"sc_{hp}_{qc}_{kc}_{sh}")
                            for hl in range(2):
                                kTv = kT_lo if hl == 0 else kT_hi
                                nc.tensor.matmul(
                                    sc[:, hl, :],
                                    kTv[:, hp,
                                        kc * 256 + sh * P: kc * 256 + (sh + 1) * P],
                                    qT[:, hp, qsl],
                                    start=True, stop=True)
                            nc.scalar.activation(
                                ex[:, sh, :, :], sc[:], ACT_F.Exp,
                                bias=0.0, scale=SCALE)
                            if kc >= 2 * qc:
                                m_i = 2 * (kc - 2 * qc) + sh
                                nc.vector.tensor_tensor(
                                    out=ex[:, sh, :, :], in0=ex[:, sh, :, :],
                                    in1=masks[m_i][:], op=ALU.mult)
                        for hl in range(2):
                            nc.tensor.matmul(
                                av[hl][:],
                                v_aug[:, 2 * kc:2 * kc + 2, 2 * hp + hl, :],
                                ex[:, :, hl, :],
                                start=(kc == 0), stop=(kc == 2 * qc + 1),
                                perf_mode=DR)
                    # normalize: recip of sum row, K=1 ones-matmul broadcast
                    for hl in range(2):
                        h = 2 * hp + hl
                        lden = st.tile([P, CH2], f32, tag="lden", bufs=2,
                                       name=f"lden_{hp}_{qc}_{hl}")
                        nc.scalar.activation(lden[D:D + 1, :],
                                             av[hl][D:D + 1, :], ACT_F.Ln)
                        rden = st.tile([P, CH2], bf16, tag="rden", bufs=2,
                                       name=f"rden_{hp}_{qc}_{hl}")
                        nc.scalar.activation(rden[D:D + 1, :],
                                             lden[D:D + 1, :], ACT_F.Exp,
                                             bias=0.0, scale=-1.0)
                        pb = ps.tile([P, CH2], f32, tag="big", bufs=2,
                                     name=f"pb_{hp}_{qc}_{hl}")
                        nc.tensor.matmul(pb[0:D, :], ones_bf[D:D + 1, 0:D],
                                         rden[D:D + 1, :],
                                         start=True, stop=True)
                        pbs = st.tile([P, CH2], bf16, tag="pbs", bufs=2,
                                      name=f"pbs_{hp}_{qc}_{hl}")
                        nc.vector.tensor_copy(out=pbs[0:D, :], in_=pb[0:D, :])
                        nc.vector.tensor_tensor(
                            out=attnT[(h % 2) * D:(h % 2 + 1) * D, h // 2, qsl],
                            in0=av[hl][0:D, :], in1=pbs[0:D, :], op=ALU.mult)

                # ---- proj/RS delayed one chunk: never head-of-line
                # blocks the next attention chunk on the in-order queues ----
                for pr in ([qc - 1] if qc < QC2 - 1 else [qc - 1, qc]):
                    if pr >= 0:
                        _emit_proj_rs(nc, st, ps, attnT, wp_sb,
                                      rs_in, rs_out, pr)

            # ---- per-chunk: residual + LN2 -> h2 -> h2T; FFN in two
            # token-halves so FFN-A starts after RS(1) and overlaps the
            # remaining RS chain (skew robustness: FFN2-A before FFN1-B) ----
            out1 = sb.tile([P, NCHUNK, C], f32, name="out1")
            h2 = sb.tile([P, NCHUNK, C], bf16, tag="h2t", name="h2")
            h2T = sb.tile([P, KT, T_OWN], bf16, name="h2T")
            s2 = sb.tile([P, 4, NCHUNK], f32, name="ln2_s")
            ff1T = sb.tile([P, FMT, T_OWN], bf16, tag="xfull", name="ff1T")
            rs_sb_all = sb.tile([P, NCHUNK, C], bf16, name="rs_sb_all")
            for r in range(NCHUNK):
                nc.sync.dma_start(rs_sb_all[:, r, :], rs_out[r][:])

            def ln2_chunk(r):
                nc.vector.tensor_tensor(out=out1[:, r, :], in0=x_own[:, r, :],
                                        in1=rs_sb_all[:, r, :], op=ALU.add)
                bn2 = st.tile([P, 2, 6], f32, tag="bn2", bufs=2,
                              name=f"bn2_{r}")
                for hh in range(2):
                    nc.vector.bn_stats(bn2[:, hh, :],
                                       out1[:, r, hh * 512:(hh + 1) * 512])
                nc.vector.bn_aggr(s2[:, 0:2, r], bn2[:])
                nc.scalar.activation(s2[:, 3, r:r + 1], s2[:, 1, r:r + 1],
                                     ACT_F.Ln, bias=eps_t[:])
                nc.scalar.activation(s2[:, 3, r:r + 1], s2[:, 3, r:r + 1],
                                     ACT_F.Exp, bias=0.0, scale=-0.5)
                m2 = st.tile([P, 1], f32, tag="m2", bufs=2, name=f"m2_{r}")
                nc.vector.tensor_mul(out=m2[:], in0=s2[:, 0, r:r + 1],
                                     in1=s2[:, 3, r:r + 1])
                nc.vector.tensor_scalar_mul(m2[:], m2[:], -1.0)
                nc.scalar.activation(
                    h2[:, r, :], out1[:, r, :], ACT_F.Identity,
                    bias=m2[:], scale=s2[:, 3, r:r + 1])
                nc.vector.tensor_tensor(out=out1[:, r, :], in0=out1[:, r, :],
                                        in1=b2_r[:], op=ALU.add)
                tp = ps.tile([P, KT * P], bf16, tag="mid", bufs=2,
                             name=f"tp_h2_{r}")
                for ct in range(KT):
                    nc.tensor.transpose(tp[:, ct * P:(ct + 1) * P],
                                        h2[:, r, ct * P:(ct + 1) * P],
                                        id_bf[:])
                nc.vector.tensor_copy(
                    out=h2T[:, :, r * P:(r + 1) * P],
                    in_=tp[:].rearrange("p (ct x) -> p ct x", x=P))

            def ffn1_half(hf):
                tsl = slice(hf * 256, hf * 256 + 256)
                for mt in range(FMT):
                    w1s = st.tile([P, KT, P], bf16, tag="w1st", bufs=3,
                                  name=f"w1st_{hf}_{mt}")
                    nc.sync.dma_start(
                        w1s[:],
                        w1_ext[mt].rearrange("p (kt m) -> p kt m", kt=KT))
                    pp = ps.tile([P, 256], f32, tag="big", bufs=2,
                                 name=f"ff1_{hf}_{mt}")
                    for kt in range(KT):
                        nc.tensor.matmul(pp[:], w1s[:, kt, :], h2T[:, kt, tsl],
                                         start=(kt == 0), stop=(kt == KT - 1))
                    nc.scalar.activation(ff1T[:, mt, tsl], pp[:], ACT_F.Relu,
                                         bias=b1_sb[:, mt:mt + 1])

            def ffn2_half(hf):
                for n in range(2):
                    accs = [ps.tile([P, 512], f32, tag="mid", bufs=2,
                                    name=f"ff2_{hf}_{n}_{m}")
                            for m in (2 * hf, 2 * hf + 1)]
                    for kt in range(FMT):
                        w2s = st.tile([P, 512], bf16, tag="w2s", bufs=4,
                                      name=f"w2s_{hf}_{n}_{kt}")
                        nc.sync.dma_start(
                            w2s[:], w2_ext[n][:, kt * 512:(kt + 1) * 512])
                        for mi, m in enumerate((2 * hf, 2 * hf + 1)):
                            nc.tensor.matmul(
                                accs[mi][:], ff1T[:, kt, m * P:(m + 1) * P],
                                w2s[:],
                                start=(kt == 0), stop=(kt == FMT - 1))
                    for mi, m in enumerate((2 * hf, 2 * hf + 1)):
                        ob = st.tile([P, 512], f32, tag="outev", bufs=2,
                                     name=f"outev_{hf}_{m}_{n}")
                        nc.vector.tensor_tensor(
                            out=ob[:], in0=accs[mi][:],
                            in1=out1[:, m, n * 512:(n + 1) * 512], op=ALU.add)
                        nc.sync.dma_start(
                            out_ext[m * P:(m + 1) * P, n * 512:(n + 1) * 512],
                            ob[:])

            ln2_chunk(0)
            ln2_chunk(1)
            ffn1_half(0)
            ffn2_half(0)
            ln2_chunk(2)
            ln2_chunk(3)
            ffn1_half(1)
            ffn2_half(1)
_NC_CACHE = None


def _get_nc():
    global _NC_CACHE
    if _NC_CACHE is None:
        _NC_CACHE = build()
    return _NC_CACHE


def shard_inputs(x, Wq, Wk, Wv, Wproj, bproj, W1, b1, W2, b2,
                 ln1_w, ln1_b, ln2_w, ln2_b):
    x = np.asarray(x, np.float32)
    f = np.float32
    Wq, Wk, Wv, Wproj = (np.asarray(a, f) for a in (Wq, Wk, Wv, Wproj))
    W1, W2 = np.asarray(W1, f), np.asarray(W2, f)
    bproj, b1, b2 = (np.asarray(a, f) for a in (bproj, b1, b2))
    ln1_w, ln1_b = np.asarray(ln1_w, f), np.asarray(ln1_b, f)
    ln2_w, ln2_b = np.asarray(ln2_w, f), np.asarray(ln2_b, f)

    # fold LN1 gamma into Wq/Wk/Wv; beta contributions:
    #   q/k get ln1_b @ W as a per-feature bias; v's goes through proj
    #   into bproj (added exactly once per token on the owning core).
    Wq_f = ln1_w[:, None] * Wq
    Wk_f = ln1_w[:, None] * Wk
    Wv_f = ln1_w[:, None] * Wv
    bq_full = ln1_b @ Wq          # [C]
    bk_full = ln1_b @ Wk
    bproj_eff = bproj + (ln1_b @ Wv) @ Wproj
    # fold LN2 gamma/beta into W1/b1
    W1_f = ln2_w[:, None] * W1
    b1_eff = b1 + ln2_b @ W1

    def qkv8(Wf, hs):   # [C, 256] -> [kp 128, kt 8, d 256] fp8 prescaled
        w = (Wf[:, hs] * WS).reshape(KT, P, DL).transpose(1, 0, 2)
        return np.ascontiguousarray(w.reshape(P, KT * DL)).astype(NP_F8)

    w1h = W1_f.reshape(KT, P, FMT, P).transpose(2, 1, 0, 3)  # [mt, kp, kt, m]
    w1h = np.ascontiguousarray(w1h.reshape(FMT, P, KT * P)).astype(NP_BF16)
    w2h = W2.reshape(FMT, P, 2, 512).transpose(2, 1, 0, 3)   # [n, kp, kt, ni]
    w2h = np.ascontiguousarray(w2h.reshape(2, P, FMT * 512)).astype(NP_BF16)
    b1r = np.ascontiguousarray(b1_eff.reshape(FMT, P).T)     # [p, mt]

    in_maps = []
    for c in range(NCORES):
        b, j = c // 4, c % 4
        hs = slice(DL * j, DL * (j + 1))
        own_rows = np.concatenate(
            [np.arange(512 * r + P * j, 512 * r + P * j + P)
             for r in range(NCHUNK)])
        # attnT row order: feature row (kt2=k, partition p) holds
        # head h = 2k + p//64, dim d = p%64 (heads 0,1 in kt2=0; 2,3 in 1)
        Wp_l = Wproj[hs, :]                                  # [256, C]
        perm = np.empty(DL, np.int64)
        for k in range(2):
            for p_i in range(P):
                h_loc = 2 * k + p_i // 64
                perm[k * P + p_i] = h_loc * D + p_i % 64
        wp = Wp_l[perm].reshape(2, P, C)
        bqk = np.stack([bq_full[hs].reshape(2, P)[0],
                        bq_full[hs].reshape(2, P)[1],
                        bk_full[hs].reshape(2, P)[0],
                        bk_full[hs].reshape(2, P)[1]], axis=1)  # [p, 4]
        in_maps.append({
            "x": np.ascontiguousarray(x[b]).astype(NP_BF16),
            "xo": np.ascontiguousarray(x[b][own_rows]
                                       + bproj_eff[None, :]).astype(NP_BF16),
            "wq": qkv8(Wq_f, hs),
            "wk": qkv8(Wk_f, hs),
            "wv": qkv8(Wv_f, hs),
            "wp": np.ascontiguousarray(wp).astype(NP_BF16),
            "w1": w1h,
            "w2": w2h,
            "bqk": np.ascontiguousarray(bqk, f),
            "b1r": b1r,
            "bpb2": np.ascontiguousarray(np.stack([bproj_eff, b2]), f),
        })
    return in_maps


def assemble(results):
    out = np.empty((2, T, C), np.float32)
    for c in range(NCORES):
        b, j = c // 4, c % 4
        for r in range(NCHUNK):
            out[b, 512 * r + P * j: 512 * r + P * j + P] = \
                results[c]["out"][r * P:(r + 1) * P]
    return out


def kernel(**inputs):
    nc = _get_nc()
    in_maps = shard_inputs(**{k: np.asarray(v) for k, v in inputs.items()})
    res = run_bass_kernel_spmd(nc, in_maps, list(range(NCORES)))
    return assemble(res.results)
